# revision 1
# baseline (speedup 1.0000x reference)
"""Trainium2 Bass kernel for circular 3x3 conv (im2col-free shift-pair GEMM).

out[b,h,w,f] = sum_{dh,dw,c} x[b,(h-dh)%H,(w-dw)%W,c] * K[j*C+c, f] + bias[f]
with j = dw_idx*3 + dh_idx, dh = [-1,0,1][dh_idx], dw = [-1,0,1][dw_idx].

Per core (8 cores, 2 batches each):
  - x loaded partition=h, free=(w,c), in 8 w-blocks of 16 cols -> 4KB
    contiguous DMA descriptors (line rate).
  - PE transposes only EVEN w-col pairs -> slab E[i] = [c(x[:,2i]) ; c(x[:,2i+1])]
    on 128 partitions, free = h (+2 circular pad cols), rounded to fp32r by
    the DVE PSUM->SBUF copy.
  - Per output col w (i = w//2), 6 fp32r matmuls accumulate PSUM [128 h, 256 f]:
      even w: 3x pair K=128 from E[i] (kernel rows [dw=0; dw=-1])
              + 3x single K=64 from E[i-1] bottom half (dw=+1, array rows 64-127)
      odd w:  3x pair K=128 from E[i] (kernel rows [dw=+1; dw=0])
              + 3x single K=64 from E[i+1] top half (dw=-1)
    DVE adds bias -> SBUF; 1KB-strided DMA out. Slab production is software-
    pipelined ahead of consumption.
"""
import numpy as np

B, H, W, C, F = 16, 128, 128, 64, 256
NCORES = 8
BPC = B // NCORES  # batches per core
NBLK = 8  # w blocks per batch
BLKW = W // NBLK  # 16 cols per block
NE = W // 2  # even slabs per batch


def _build_module(reps=1):
    import concourse.bacc as bacc
    import concourse.mybir as mybir
    import concourse.tile as tile

    f32 = mybir.dt.float32
    f32r = mybir.dt.float32r

    nc = bacc.Bacc("TRN2", target_bir_lowering=False, debug=False,
                   num_devices=NCORES)
    xc_d = nc.dram_tensor("xc", [BPC, H, W, C], f32, kind="ExternalInput").ap()
    kw_d = nc.dram_tensor("kw", [9 * C, F], f32, kind="ExternalInput").ap()
    biasf_d = nc.dram_tensor("biasf", [128, F], f32, kind="ExternalInput").ap()
    ident_d = nc.dram_tensor("ident", [128, 128], f32, kind="ExternalInput").ap()
    out_d = nc.dram_tensor("out", [BPC, H, W, F], f32, kind="ExternalOutput").ap()

    with tile.TileContext(nc) as tc:
        with (
            tc.tile_pool(name="persist", bufs=1) as persist,
            tc.tile_pool(name="kraw", bufs=2) as kraw_pool,
            tc.tile_pool(name="slab_sb", bufs=7) as slab_pool,
            tc.tile_pool(name="out_sb", bufs=4) as out_pool,
            tc.tile_pool(name="ps_slab", bufs=3, space="PSUM") as ps_slab,
            tc.tile_pool(name="ps_out", bufs=5, space="PSUM") as ps_out,
        ):
            # ---- static prep: identity first (gates transposes), then
            # starters, kernel tiles, bias ----
            ident = persist.tile([128, 128], f32, tag="ident")
            nc.sync.dma_start(ident[:], ident_d[:])

            start_a = persist.tile([H, 2, C], f32, tag="start_a")  # cols 126,127
            nc.scalar.dma_start(start_a[:], xc_d[0, :, W - 2:W, :])
            start_b = persist.tile([H, 4, C], f32, tag="start_b")  # cols 0..3
            nc.scalar.dma_start(start_b[:], xc_d[0, :, 0:4, :])

            # Each tile group's top/bottom halves are contiguous 192-row
            # kernel ranges -> 2 DMAs per group, 6 total.
            # kw viewed as [9, C, F]; group tile [128, 3, F]:
            #   [0:C, dhi, :] = shift j_top+dhi, [C:2C, dhi, :] = j_bot+dhi.
            kw3 = kw_d.rearrange("(j c) f -> j c f", c=C)

            def kload3(j_top, j_bot, tag, ei):
                raw = kraw_pool.tile([128, 3, F], f32, tag=f"kraw{tag}")
                _keng = [nc.sync, nc.scalar]
                _keng[ei].dma_start(
                    raw[0:C, :, :],
                    kw3[j_top:j_top + 3, :, :].rearrange("j c f -> c j f"))
                _keng[1 - ei].dma_start(
                    raw[C:2 * C, :, :],
                    kw3[j_bot:j_bot + 3, :, :].rearrange("j c f -> c j f"))
                t = persist.tile([128, 3, F], f32r, tag=tag)
                nc.vector.tensor_copy(t[:], raw[:])
                return t

            kp1_all = kload3(3, 0, "kp1", 0)  # top j=3+dhi, bottom j=dhi
            ks_all = kload3(0, 6, "ks", 1)    # top j=dhi (lo), bottom j=6+dhi (hi)
            kp2_all = kload3(6, 3, "kp2", 0)  # top j=6+dhi, bottom j=3+dhi
            kp1 = [kp1_all[:, d, :] for d in range(3)]
            kp2 = [kp2_all[:, d, :] for d in range(3)]
            ks_lo = [ks_all[:, d, :] for d in range(3)]
            ks_hi = ks_lo

            biasf = persist.tile([128, F], f32, tag="biasf")
            nc.sync.dma_start(biasf[:], biasf_d[:])

            # ---- x loads: per batch, 3 DMAs: cols 112-127 (E63 first),
            # cols 0-15, cols 16-111 ----
            xb_t = []
            for b in range(BPC):
                t7 = persist.tile([H, 16, C], f32, tag=f"xb{b}_hi")
                nc.sync.dma_start(t7[:], xc_d[b, :, 112:128, :])
                t0 = persist.tile([H, 16, C], f32, tag=f"xb{b}_lo")
                nc.scalar.dma_start(t0[:], xc_d[b, :, 0:16, :])
                tms = []
                for ci in range(4):
                    w0 = 16 + 24 * ci
                    tm = persist.tile([H, 24, C], f32, tag=f"xb{b}_m{ci}")
                    eng = nc.sync if ci % 2 == 0 else nc.scalar
                    eng.dma_start(tm[:], xc_d[b, :, w0:w0 + 24, :])
                    tms.append(tm)
                xb_t.append((t0, tms, t7))

            # ---- main loop ----
            def make_eslab(b, i, tag, src=None, src_col=0):
                """Transpose cols (2i, 2i+1) -> slab [128, H+2] fp32r."""
                if src is None:
                    w0 = 2 * i
                    t0, tms, t7 = xb_t[b]
                    if w0 < 16:
                        src, src_col = t0, w0
                    elif w0 < 112:
                        src, src_col = tms[(w0 - 16) // 24], (w0 - 16) % 24
                    else:
                        src, src_col = t7, w0 - 112
                ps = ps_slab.tile([128, H], f32, tag="pslab")
                nc.tensor.matmul(ps[:], src[:, src_col:src_col + 2, :], ident[:],
                                 is_transpose=True, start=True, stop=True)
                sl = slab_pool.tile([128, H + 2], f32r, tag=tag)
                nc.vector.tensor_copy(sl[:, 1:H + 1], ps[:])
                nc.vector.tensor_copy(sl[:, 0:1], ps[:, H - 1:H])
                nc.vector.tensor_copy(sl[:, H + 1:H + 2], ps[:, 0:1])
                return sl

            for _rep in range(reps):
              for b in range(BPC):
                if b == 0 and _rep == 0:
                    E = {NE - 1: make_eslab(b, NE - 1, tag="e63",
                                            src=start_a, src_col=0),
                         0: make_eslab(b, 0, tag="e0", src=start_b, src_col=0),
                         1: make_eslab(b, 1, tag="slab", src=start_b, src_col=2)}
                else:
                    E = {NE - 1: make_eslab(b, NE - 1, tag="e63"),
                         0: make_eslab(b, 0, tag="e0"),
                         1: make_eslab(b, 1, tag="slab")}
                for w in range(W):
                    i = w // 2
                    po = ps_out.tile([H, F], f32, tag="pout")
                    if w % 2 == 0:
                        pair_sl, pair_k = E[i], kp1
                        sng = E[(i - 1) % NE]
                        sng_lo, sng_hi, sng_k = C, 2 * C, ks_hi
                    else:
                        pair_sl, pair_k = E[i], kp2
                        sng = E[(i + 1) % NE]
                        sng_lo, sng_hi, sng_k = 0, C, ks_lo
                    for dhi in range(3):
                        off = 2 - dhi  # dh = [-1,0,1][dhi] -> off = 1-dh
                        nc.tensor.matmul(
                            po[:], pair_sl[:, off:off + H], pair_k[dhi][:],
                            start=(dhi == 0), stop=False,
                        )
                        nc.tensor.matmul(
                            po[:], sng[sng_lo:sng_hi, off:off + H],
                            sng_k[dhi][sng_lo:sng_hi, :],
                            start=False, stop=(dhi == 2),
                        )
                    if w % 4 == 0:
                        ob = out_pool.tile([H, 4, F], f32, tag="outsb")
                        ob_quad = ob
                    else:
                        ob = ob_quad
                    nc.vector.tensor_add(ob[:, w % 4, :], po[:], biasf[:])
                    if w % 4 == 3:
                        eng = nc.sync if w % 8 == 3 else nc.scalar
                        eng.dma_start(out_d[b, :, w - 3:w + 1, :], ob[:])
                    # software pipeline: produce E[i+2] at even steps
                    if w % 2 == 0 and i + 2 <= NE - 2:
                        E[i + 2] = make_eslab(b, i + 2, tag="slab")
                    if w % 2 == 1 and i >= 2:
                        E.pop(i - 1, None)

    nc.compile()
    return nc


_NC_CACHE = {}


def _get_nc(reps=1):
    if reps not in _NC_CACHE:
        _NC_CACHE[reps] = _build_module(reps)
    return _NC_CACHE[reps]


def kernel(x, kernel, bias, _trace=False):
    from concourse.bass_utils import run_bass_kernel_spmd

    x = np.ascontiguousarray(np.asarray(x, dtype=np.float32))
    kern = np.ascontiguousarray(np.asarray(kernel, dtype=np.float32))
    bias = np.asarray(bias, dtype=np.float32)
    biasf = np.ascontiguousarray(np.broadcast_to(bias[None, :], (128, F)))
    ident = np.eye(128, dtype=np.float32)

    nc = _get_nc()
    in_maps = [
        {"xc": x[c * BPC:(c + 1) * BPC], "kw": kern, "biasf": biasf,
         "ident": ident}
        for c in range(NCORES)
    ]
    res = run_bass_kernel_spmd(nc, in_maps, core_ids=list(range(NCORES)),
                               trace=_trace)
    out = np.concatenate([res.results[c]["out"] for c in range(NCORES)], axis=0)
    if _trace:
        kernel._last_results = res
    return out



# revision 15
# speedup vs baseline: 1.3503x; 1.3503x over previous
"""Trainium2 Bass kernel for circular 3x3 conv — host-packed bf16 GEMM form.

out[b,h,w,f] = sum_{dh,dw,c} x[b,(h-dh)%H,(w-dw)%W,c] * K[j*C+c, f] + bias[f]
with j = dhi + 3*dwi, dh = dhi-1, dw = dwi-1.

Strategy (cost-model-driven): every matmul costs out_free_size cycles on the
PE regardless of contraction depth, so pack the 9*C=576-deep contraction into
as few, widest matmuls as possible. Host pre-packs x into three bf16 layouts
so that each output column PAIR (w=2i, 2i+1) accumulates in one PSUM tile
[128h, 512=(2 cols x 256f)] with exactly 6 matmuls (4x N=512 + 2x N=256):

  MM1-3 (d=dhi): lhsT = xe[i] slab (cols 2i,2i+1 on partitions, padded h
         free) sliced at h-offset 2-d; rhs = T_d [128,512] covering
         (dw=0,-1) for col w and (dw=+1,0) for col w+1 — all 4 quadrants
         of the kernel tile are live.
  MM4:   lhsT = xq[2i-1]  (col w-1 at dh=-1,0 stacked); rhs = [K_6;K_7]
         -> po[:,0:256]   (col w's dw=+1 taps, dhi=0,1)
  MM5:   lhsT = xq[2i+2]  (col w+2);  rhs = [K_0;K_1] -> po[:,256:512]
  MM6:   lhsT = xr[i] = [col 2i-1 @ dh=+1 ; col 2i+2 @ dh=+1];
         rhs = [[K_8,0],[0,K_2]] (zero-masked) -> po[:,0:512]

DVE adds bias and casts to bf16; bulk 8-column DMAs write out. Host casts
the bf16 result back to fp32. Inputs/kernel in bf16 (rel err ~4e-4 << 2e-2).
"""
import numpy as np

B, H, W, C, F = 16, 128, 128, 64, 256
NCORES = 8
BPC = B // NCORES   # batches per core
NP = W // 2         # column pairs per batch
NCHUNK = 8          # input streaming chunks per batch


def _build_module():
    import concourse.bacc as bacc
    import concourse.mybir as mybir
    import concourse.tile as tile

    f32 = mybir.dt.float32
    bf16 = mybir.dt.bfloat16

    nc = bacc.Bacc("TRN2", target_bir_lowering=False, debug=False,
                   num_devices=NCORES)
    xe_d = nc.dram_tensor("xe", [BPC, 128, NP, 130], bf16,
                          kind="ExternalInput").ap()
    xq_d = nc.dram_tensor("xq", [BPC, 128, W, 128], bf16,
                          kind="ExternalInput").ap()
    xr_d = nc.dram_tensor("xr", [BPC, 128, NP, 128], bf16,
                          kind="ExternalInput").ap()
    # x0 packs chunk 0 of xe/xq/xr (first 4 pairs) into one fast DMA:
    # [4x130 xe | 8x128 xq | 4x128 xr] = 2056 per partition.
    x0_d = nc.dram_tensor("x0", [BPC, 128, 2056], bf16,
                          kind="ExternalInput").ap()
    kt_d = nc.dram_tensor("kt", [128, 6, 512], bf16, kind="ExternalInput").ap()
    biasf_d = nc.dram_tensor("biasf", [128, 512], f32,
                             kind="ExternalInput").ap()
    out_d = nc.dram_tensor("out", [BPC, H, W, F], bf16,
                           kind="ExternalOutput").ap()

    # Geometric chunk boundaries (slab index space): chunk 0 lives in x0;
    # later chunks stream from xe/xq/xr. xq is in column space (2x).
    ECH = [4, 16, 40, 64]
    QCH = [8, 32, 80, 128]
    WARMUP = 0

    with tile.TileContext(nc) as tc:
        with (
            tc.tile_pool(name="persist", bufs=1) as persist,
            tc.tile_pool(name="xdbl", bufs=2) as xdbl,
            tc.tile_pool(name="outp", bufs=3) as outp,
            tc.tile_pool(name="ps", bufs=6, space="PSUM") as ps,
            tc.tile_pool(name="psw", bufs=1, space="PSUM") as psw,
        ):
            kt = persist.tile([128, 6, 512], bf16, tag="kt")
            nc.sync.dma_start(kt[:], kt_d[:])
            if WARMUP:
                # p-state warmup matmuls (disabled: the cost model's PE
                # busy-tracker interacts badly with a warmup stream).
                warm = psw.tile([32, 32], f32, tag="warm")
                for _ in range(WARMUP):
                    nc.tensor.matmul(warm[:], kt[:, 0, 0:32], kt[:, 0, 32:64],
                                     start=True, stop=True)

            NCH = len(ECH) - 1
            x0_sb = [None] * BPC
            xe_sb = [[None] * NCH for _ in range(BPC)]
            xq_sb = [[None] * NCH for _ in range(BPC)]
            xr_sb = [[None] * NCH for _ in range(BPC)]

            def load_chunk(b, g):
                pool = xdbl if g < 1 else persist
                e0, e1 = ECH[g], ECH[g + 1]
                q0, q1 = QCH[g], QCH[g + 1]
                te = pool.tile([128, e1 - e0, 130], bf16, tag=f"xe_{g}")
                nc.sync.dma_start(te[:], xe_d[b, :, e0:e1, :])
                tq = pool.tile([128, q1 - q0, 128], bf16, tag=f"xq_{g}")
                nc.sync.dma_start(tq[:], xq_d[b, :, q0:q1, :])
                tr = pool.tile([128, e1 - e0, 128], bf16, tag=f"xr_{g}")
                nc.sync.dma_start(tr[:], xr_d[b, :, e0:e1, :])
                xe_sb[b][g] = te
                xq_sb[b][g] = tq
                xr_sb[b][g] = tr

            # Input DMA issue order: batch-0 first pairs ASAP, then batch-0
            # bulk, batch-1 interleaved behind. Early chunks double-buffered
            # so batch-1's loads run ahead; big chunks (2,3) single-buffered
            # so their batch-1 DMAs self-throttle on batch-0's readers
            # (keeps the DMA device free for output writes mid-flight).
            biasf = persist.tile([128, 512], f32, tag="biasf")
            nc.sync.dma_start(biasf[:], biasf_d[:])
            t0 = xdbl.tile([128, 2056], bf16, tag="x0")
            nc.sync.dma_start(t0[:], x0_d[0, :, :])
            x0_sb[0] = t0
            load_chunk(0, 0)
            load_chunk(0, 1)
            load_chunk(0, 2)
            t1 = xdbl.tile([128, 2056], bf16, tag="x0")
            nc.sync.dma_start(t1[:], x0_d[1, :, :])
            x0_sb[1] = t1
            load_chunk(1, 0)
            load_chunk(1, 1)
            load_chunk(1, 2)

            def eslab(b, i):
                if i < 4:
                    return x0_sb[b][:, i * 130:(i + 1) * 130]
                for g in range(NCH):
                    if i < ECH[g + 1]:
                        return xe_sb[b][g][:, i - ECH[g], :]
                raise AssertionError(i)

            def qslab(b, u):
                if u < 8:
                    return x0_sb[b][:, 520 + u * 128:520 + (u + 1) * 128]
                for g in range(NCH):
                    if u < QCH[g + 1]:
                        return xq_sb[b][g][:, u - QCH[g], :]
                raise AssertionError(u)

            def rslab(b, i):
                if i < 4:
                    return x0_sb[b][:, 1544 + i * 128:1544 + (i + 1) * 128]
                for g in range(NCH):
                    if i < ECH[g + 1]:
                        return xr_sb[b][g][:, i - ECH[g], :]
                raise AssertionError(i)

            # Pair processing order: 1..56 in 8-pair groups (16-col DMAs),
            # then 57-60, 61-62, 63, 0 (wrap columns). Batch 1 runs the
            # wrap pairs mid-stream (its late chunks are resident by then)
            # so the kernel tail ends on small DMAs.
            groups = [list(range(1 + 8 * g, 9 + 8 * g)) for g in range(7)]
            groups.append([57, 58, 59, 60])
            groups.append([61])
            groups.append([62])
            groups.append([63])
            groups.append([0])
            b1_groups = groups

            for b in range(BPC):
                for gi, grp in enumerate(groups if b == 0 else b1_groups):
                    ob = outp.tile([128, 16, F], bf16, tag="ob")
                    for j, i in enumerate(grp):
                        po = ps.tile([128, 512], f32, tag="po")
                        e_i = eslab(b, i)
                        for d in range(3):
                            off = 2 - d
                            nc.tensor.matmul(po[:], e_i[:, off:off + 128],
                                             kt[:, d, :],
                                             start=(d == 0), stop=False)
                        nc.tensor.matmul(po[:, 0:256],
                                         qslab(b, (2 * i - 1) % W),
                                         kt[:, 3, 0:256],
                                         start=False, stop=False)
                        nc.tensor.matmul(po[:, 256:512],
                                         qslab(b, (2 * i + 2) % W),
                                         kt[:, 4, 0:256],
                                         start=False, stop=False)
                        nc.tensor.matmul(po[:], rslab(b, i), kt[:, 5, :],
                                         start=False, stop=True)
                        nc.vector.tensor_add(ob[:, 2 * j:2 * j + 2, :],
                                             po[:], biasf[:])
                    w0 = 2 * grp[0]
                    nc.scalar.dma_start(out_d[b, :, w0:w0 + 2 * len(grp), :],
                                        ob[:, 0:2 * len(grp), :])

    nc.compile()
    return nc


_NC_CACHE = {}


def _get_nc():
    if "nc" not in _NC_CACHE:
        _NC_CACHE["nc"] = _build_module()
    return _NC_CACHE["nc"]


def _pack_inputs(x, kern, bias):
    import ml_dtypes

    bf16 = ml_dtypes.bfloat16
    x = np.asarray(x, dtype=np.float32)
    kern = np.asarray(kern, dtype=np.float32)
    bias = np.asarray(bias, dtype=np.float32)

    xt = np.transpose(x, (0, 2, 3, 1))          # [B, W, C, H]
    # xe: [B, p=(wp*64+c), i, h'] with h' = 130 circularly padded h
    xpad = np.concatenate([xt[..., H - 1:H], xt, xt[..., 0:1]], axis=-1)
    xe = (xpad.reshape(B, NP, 2, C, 130)
          .transpose(0, 2, 3, 1, 4)
          .reshape(B, 128, NP, 130)).astype(bf16)
    # xq: [B, p=(s*64+c), u, h], s=0 -> dh=-1 (x[h+1]), s=1 -> dh=0
    xm1 = np.roll(xt, -1, axis=-1)              # x[(h+1)%H]
    xq = (np.stack([xm1, xt], axis=1)           # [B, 2, W, C, H]
          .transpose(0, 1, 3, 2, 4)
          .reshape(B, 128, W, 128)).astype(bf16)
    # xr: [B, p, i, h]: rows 0:64 = col (2i-1)%W @ dh=+1, 64:128 = (2i+2)%W
    xp1 = np.roll(xt, 1, axis=-1)               # x[(h-1)%H]
    i_arr = np.arange(NP)
    top = xp1[:, (2 * i_arr - 1) % W].transpose(0, 2, 1, 3)   # [B, C, NP, H]
    bot = xp1[:, (2 * i_arr + 2) % W].transpose(0, 2, 1, 3)
    xr = np.concatenate([top, bot], axis=1).astype(bf16)      # [B, 128, NP, H]

    # kernel tiles [128, 6, 512]
    kw3 = kern.reshape(9, C, F)
    kt = np.zeros((6, 128, 512), dtype=np.float32)
    for d in range(3):
        kt[d, 0:64, 0:256] = kw3[d + 3]
        kt[d, 64:128, 0:256] = kw3[d]
        kt[d, 0:64, 256:512] = kw3[d + 6]
        kt[d, 64:128, 256:512] = kw3[d + 3]
    kt[3, 0:64, 0:256] = kw3[6]
    kt[3, 64:128, 0:256] = kw3[7]
    kt[4, 0:64, 0:256] = kw3[0]
    kt[4, 64:128, 0:256] = kw3[1]
    kt[5, 0:64, 0:256] = kw3[8]
    kt[5, 64:128, 256:512] = kw3[2]
    kt = np.ascontiguousarray(kt.transpose(1, 0, 2)).astype(bf16)

    biasf = np.ascontiguousarray(
        np.broadcast_to(np.tile(bias, 2)[None, :], (128, 512))).astype(
            np.float32)
    x0 = np.concatenate([xe[:, :, 0:4, :].reshape(B, 128, 520),
                         xq[:, :, 0:8, :].reshape(B, 128, 1024),
                         xr[:, :, 0:4, :].reshape(B, 128, 512)],
                        axis=-1)
    return xe, xq, xr, x0, kt, biasf


def kernel(x, kernel, bias, _trace=False):
    from concourse.bass_utils import run_bass_kernel_spmd

    xe, xq, xr, x0, kt, biasf = _pack_inputs(x, kernel, bias)

    nc = _get_nc()
    in_maps = [
        {"xe": np.ascontiguousarray(xe[c * BPC:(c + 1) * BPC]),
         "xq": np.ascontiguousarray(xq[c * BPC:(c + 1) * BPC]),
         "xr": np.ascontiguousarray(xr[c * BPC:(c + 1) * BPC]),
         "x0": np.ascontiguousarray(x0[c * BPC:(c + 1) * BPC]),
         "kt": kt, "biasf": biasf}
        for c in range(NCORES)
    ]
    res = run_bass_kernel_spmd(nc, in_maps, core_ids=list(range(NCORES)),
                               trace=_trace)
    out = np.concatenate([np.asarray(res.results[c]["out"])
                          for c in range(NCORES)], axis=0)
    if _trace:
        kernel._last_results = res
    return out.astype(np.float32)
